# revision 28
# baseline (speedup 1.0000x reference)
"""Trainium2 Bass kernel for dense_cnn problem.

Math (per batch element n, C=128 channels, H=W=56, G=8):
  t1 = conv_h(x, w1)          5-tap conv over H with full channel mixing
  t3 = dwconv_h(t1, w3)       3-tap depthwise conv over H
  t4[g] = sum_{c,k} x[c, h, w+2k-2] * w4[c,k,g]   (3 width taps, dil 2)
  out[c] = t3[c] * t4[c % 8]

Device strategy (data-parallel, 4 batch elems per core across 8 cores):
  - Fold t3 = w3 (*) w1 (*) x into ONE 7-tap H-conv with combined
    weights -> PE matmuls (plus 4 tiny correction matmuls for the rows
    where the fold wrongly includes clipped t1 values).  Measured DVE
    elementwise costs make this strictly better than computing the
    depthwise conv on the vector engines: the fold costs 2 extra PE
    taps (~373ns/chunk) vs ~2500ns/chunk of elementwise work.
  - w4 broadcast to 128 output channels on the host -> t4 computed at
    128 channels (3 PE matmuls), final combine elementwise.
  - Matmuls in bf16; accumulation fp32 in PSUM.  x unpadded in SBUF;
    boundary taps are clipped matmuls.
  - Per 8-row chunk: psA (t3) and psB (t4); ScalarE copies psB->SBUF
    (bf16), DVE multiplies psA (fp32 PSUM) against it writing the bf16
    output strip.  Output DMA'd bf16 (half the bytes of fp32) and
    upconverted on the host; one DMA per element, the last element
    split so only a single 8-row transfer trails the last multiply.
  - x loads for elements 1-3 are issued mid-stream: issuing them up
    front steals HBM bandwidth from the pieces the PE needs first, and
    the resulting matmul stalls also reset the PE_HAM activity window,
    delaying the 1.2 -> 2.4 GHz clock boost.
  - Small dummy matmuls (sized to end right as the first x piece and
    weights land) trip the PE_HAM clock gate while the first DMAs are
    still streaming in.
"""

import sys

sys.path.insert(0, "/opt/trn_rl_repo")

import ml_dtypes
import numpy as np

import concourse.bacc as bacc
import concourse.bass as bass
import concourse.mybir as mybir
import concourse.tile as tile
from concourse import bass_utils

N, C, H, W, G = 32, 128, 56, 56, 8
NCORES = 8
NPC = N // NCORES  # batch elems per core
CH = 8             # H rows per chunk (PSUM bank = 448 fp32 cols)
NCHUNK = H // CH

F32 = mybir.dt.float32
BF16 = mybir.dt.bfloat16

TRACE = False
TRACE_DIR = None
LAST_EXEC_NS = None
LAST_RESULTS = None

_COMPILED = None


def _enable_trace_hook():
    """The agent image's ``antenv`` lacks ``axon_hooks``, so the boot-time
    NTFF hook registration silently degraded. Recreate the module and
    register the same ctypes-based hook; also skip the bucket upload."""
    import sys as _sys
    import types

    if "antenv.axon_hooks" not in _sys.modules:
        mod = types.ModuleType("antenv.axon_hooks")
        mod._hook = None

        def set_axon_ntff_profile_hook(h):
            mod._hook = h

        def get_axon_ntff_profile_hook():
            return mod._hook

        mod.set_axon_ntff_profile_hook = set_axon_ntff_profile_hook
        mod.get_axon_ntff_profile_hook = get_axon_ntff_profile_hook
        _sys.modules["antenv.axon_hooks"] = mod
        import antenv

        antenv.axon_hooks = mod

    from antenv.axon_hooks import get_axon_ntff_profile_hook as _get

    if _get() is None:
        from trn_agent_boot.trn_boot import _ntff_profile_via_ctypes

        hook = _ntff_profile_via_ctypes("/opt/axon/libaxon_pjrt.so")
        if hook is not None:
            _sys.modules["antenv.axon_hooks"].set_axon_ntff_profile_hook(hook)

    bass_utils.upload_artifacts = lambda tmpdir: f"local:{tmpdir}"


FOLD0 = 47         # fold region rows FOLD0..H-1; t1/DVE covers 0..FOLD0-1
FROWS = H - FOLD0  # 9 rows -> 504 fp32 cols, fits one PSUM bank


def _t3_fold_matmuls(pa, xc, wc_t, h0, rows):
    """Folded 7-tap conv for t3 rows h0..h0+rows-1 (row-clipped at the H
    borders) plus the t1-clip correction taps where the region touches
    h=0 or h=H-1.  Output row o reads x row h0+o+f-3."""
    mms = []
    # f=3 covers the full chunk -> emitted first (start=True)
    for f in (3, 0, 1, 2, 4, 5, 6):
        o_lo = max(0, 3 - f - h0)
        o_hi = min(rows, H + 3 - f - h0)
        if o_lo >= o_hi:
            continue
        r0 = h0 + o_lo + f - 3
        r1 = h0 + o_hi + f - 3
        mms.append((wc_t[:, f, :], xc[:, r0:r1, :], pa[:, o_lo:o_hi, :]))
    if h0 == 0:
        # fold wrongly includes t1[-1] at h=0
        for j in range(2):
            mms.append((wc_t[:, 7 + j, :], xc[:, j : j + 1, :], pa[:, 0:1, :]))
    if h0 + rows == H:
        # fold wrongly includes t1[56] at h=55
        for j in range(2):
            mms.append(
                (wc_t[:, 9 + j, :], xc[:, 54 + j : 55 + j, :], pa[:, rows - 1 : rows, :])
            )
    return mms


def _t1_matmuls(c, pa, xc, w5_t):
    """5-tap t1 conv for the 8-row chunk c (t1 rows 8c..8c+7), clipped at
    the top border.  Tap e=2 covers the full chunk and is emitted first."""
    h0 = c * CH
    mms = []
    for e in (2, 0, 1, 3, 4):
        o_lo = max(0, 2 - e - h0)
        o_hi = min(CH, H + 2 - e - h0)
        if o_lo >= o_hi:
            continue
        r0 = h0 + o_lo + e - 2
        r1 = h0 + o_hi + e - 2
        mms.append((w5_t[:, e, :], xc[:, r0:r1, :], pa[:, o_lo:o_hi, :]))
    return mms


def _t4_matmuls(c, pb, xc, w4_t):
    """t4 chunk: 3 width taps at offsets -2/0/+2, col-clipped at borders."""
    h0 = c * CH
    rows = xc[:, h0 : h0 + CH, :]
    return [
        (w4_t[:, 1, :], rows, pb[:]),                               # delta = 0
        (w4_t[:, 0, :], xc[:, h0 : h0 + CH, 0 : W - 2], pb[:, :, 2:W]),   # -2
        (w4_t[:, 2, :], xc[:, h0 : h0 + CH, 2:W], pb[:, :, 0 : W - 2]),   # +2
    ]


def _build():
    nc = bacc.Bacc(
        "TRN2",
        target_bir_lowering=False,
        debug=False,
        enable_asserts=False,
        num_devices=NCORES,
    )

    x_d = nc.dram_tensor("x_s", (NPC, C, H, W), BF16, kind="ExternalInput").ap()
    wc_d = nc.dram_tensor("wc", (C, 11, C), BF16, kind="ExternalInput").ap()
    w5_d = nc.dram_tensor("wc5", (C, 5, C), BF16, kind="ExternalInput").ap()
    w4_d = nc.dram_tensor("w4b", (C, 3, C), BF16, kind="ExternalInput").ap()
    w3_d = nc.dram_tensor("w3c", (C, 3), F32, kind="ExternalInput").ap()
    out_d = nc.dram_tensor("out", (NPC, C, H, W), BF16, kind="ExternalOutput").ap()

    NT1 = FOLD0 // CH + 1  # 6 t1 chunks covering rows 0..47
    # depthwise units (out-row range, ready-after-t1-chunk)
    units_mid = ((0, 24, 3), (24, FOLD0, 5))

    with tile.TileContext(nc) as tc:
        with (
            tc.tile_pool(name="wpool", bufs=1) as wpool,
            tc.tile_pool(name="xpool", bufs=1) as xpool,
            tc.tile_pool(name="t1pool", bufs=2) as t1pool,
            tc.tile_pool(name="t4pool", bufs=2) as t4pool,
            tc.tile_pool(name="tmpool", bufs=2) as tmpool,
            tc.tile_pool(name="opool", bufs=2) as opool,
            tc.tile_pool(name="psA", bufs=4, space="PSUM") as papool,
            tc.tile_pool(name="psB", bufs=3, space="PSUM") as pbpool,
            tc.tile_pool(name="psD", bufs=1, space="PSUM") as pdpool,
        ):
            # Dummy matmuls on a zeroed SBUF strip while the first DMAs
            # stream in: PE_HAM ungates the 2.4 GHz clock only after
            # ~3.4us of sustained activity.  The garbage results go to a
            # PSUM bank that is never read.  memset on GpSimd (an
            # early-ready engine the PE never waits for).
            dmy = wpool.tile([C, 448], BF16)
            nc.gpsimd.memset(dmy[:], 0.0)
            dps = pdpool.tile([C, 448], F32)
            for _ in range(8):
                nc.tensor.matmul(
                    dps[:], lhsT=dmy[:, 0:C], rhs=dmy[:], start=True, stop=True
                )

            wc_t = wpool.tile([C, 11, C], BF16)
            w5_t = wpool.tile([C, 5, C], BF16)
            w4_t = wpool.tile([C, 3, C], BF16)
            w3_t = wpool.tile([C, 3], F32)

            xcs = []
            for n in range(NPC):
                xc = xpool.tile([C, H, W], BF16, name=f"xc{n}")
                xcs.append(xc)
            # weights + first batch elem first (in pieces, so chunk-0
            # matmuls start early); later elems' loads are issued from
            # inside the chunk loop
            nc.sync.dma_start(w5_t[:], w5_d[:])
            nc.sync.dma_start(xcs[0][:, 0:18, :], x_d[0, :, 0:18, :])
            nc.sync.dma_start(w4_t[:], w4_d[:])
            nc.sync.dma_start(w3_t[:], w3_d[:])
            nc.sync.dma_start(xcs[0][:, 18:34, :], x_d[0, :, 18:34, :])
            nc.sync.dma_start(xcs[0][:, 34:H, :], x_d[0, :, 34:H, :])
            nc.sync.dma_start(wc_t[:], wc_d[:])

            def emit_mms(mms):
                for i, (lhsT, rhs, outap) in enumerate(mms):
                    nc.tensor.matmul(
                        outap, lhsT=lhsT, rhs=rhs,
                        start=(i == 0), stop=(i == len(mms) - 1),
                    )

            # elements 0..NPC-2: hybrid -- plain t1 chunks for rows
            # 0..FOLD0-1 with the depthwise conv on DVE (mid-stream,
            # plenty of pipeline room), folded t3 for the last rows
            for n in range(NPC - 1):
                xc = xcs[n]

                # t1 strip rows 0..48: strip row r holds t1 row r-1; row 0
                # is the depthwise conv's zero padding
                t1s = t1pool.tile([C, FOLD0 + 2, W], BF16, name="t1s")
                nc.gpsimd.memset(t1s[:, 0:1, :], 0.0)
                t4s = t4pool.tile([C, H, W], BF16, name="t4s")
                ot = opool.tile([C, H, W], BF16, name="ot")

                def macmul(unit):
                    # t3 rows r0..r1-1 = sum of three per-partition-scaled
                    # shifted t1 reads (tensor_scalar hits the DVE fast
                    # bf16 mode; scalar_tensor_tensor would not), then the
                    # combine against the t4 strip on the Pool engine
                    r0, r1, _ = unit
                    rr = r1 - r0
                    sa = tmpool.tile([C, 24, W], BF16, name="sa")
                    sb = tmpool.tile([C, 24, W], BF16, name="sb")
                    sc = tmpool.tile([C, 24, W], BF16, name="sc")
                    uu = tmpool.tile([C, 24, W], BF16, name="uu")
                    t3 = tmpool.tile([C, 24, W], BF16, name="t3")
                    nc.vector.tensor_scalar_mul(
                        sa[:, 0:rr, :], t1s[:, r0 : r0 + rr, :], w3_t[:, 0:1]
                    )
                    nc.vector.tensor_scalar_mul(
                        sb[:, 0:rr, :], t1s[:, r0 + 1 : r0 + 1 + rr, :], w3_t[:, 1:2]
                    )
                    nc.vector.tensor_scalar_mul(
                        sc[:, 0:rr, :], t1s[:, r0 + 2 : r0 + 2 + rr, :], w3_t[:, 2:3]
                    )
                    nc.vector.tensor_add(uu[:, 0:rr, :], sa[:, 0:rr, :], sb[:, 0:rr, :])
                    nc.vector.tensor_add(t3[:, 0:rr, :], uu[:, 0:rr, :], sc[:, 0:rr, :])
                    nc.gpsimd.tensor_mul(
                        ot[:, r0:r1, :], t3[:, 0:rr, :], t4s[:, r0:r1, :]
                    )

                for c in range(NT1):
                    # single shape/name so U-chunks and the fold chunk
                    # share one 3-buffer PSUM rotation (9 rows <= 1 bank)
                    pa = papool.tile([C, FROWS, W], F32, name="pa")
                    emit_mms(_t1_matmuls(c, pa, xc, w5_t))
                    pb = pbpool.tile([C, CH, W], F32)
                    emit_mms(_t4_matmuls(c, pb, xc, w4_t))
                    # later elements' x loads, issued mid-stream (before
                    # the out-DMA issues: SyncE executes in order and
                    # the out issues block on compute semaphores)
                    if (n, c) in ((0, 2), (0, 5), (1, 5)):
                        k = n + 1 if c == 2 else n + 2
                        nc.sync.dma_start(xcs[k][:], x_d[k])
                    # PSUM chunks -> bf16 SBUF strips on ScalarE
                    nc.scalar.copy(
                        t1s[:, c * CH + 1 : c * CH + 1 + CH, :], pa[:, 0:CH, :]
                    )
                    nc.scalar.copy(t4s[:, c * CH : (c + 1) * CH, :], pb[:])
                    for unit in units_mid:
                        if unit[2] == c:
                            macmul(unit)

                # fold region rows FOLD0..H-1: 7-tap folded t3 in one
                # PSUM chunk, multiplied directly against the t4 strip
                paf = papool.tile([C, FROWS, W], F32, name="pa")
                emit_mms(_t3_fold_matmuls(paf, xc, wc_t, FOLD0, FROWS))
                pb = pbpool.tile([C, CH, W], F32)
                emit_mms(_t4_matmuls(NCHUNK - 1, pb, xc, w4_t))
                nc.scalar.copy(t4s[:, (NCHUNK - 1) * CH : H, :], pb[:])
                nc.vector.tensor_mul(
                    ot[:, FOLD0:H, :], paf[:], t4s[:, FOLD0:H, :]
                )
                nc.sync.dma_start(out_d[n], ot[:])

            # last element: depthwise on DVE only for rows 0..38 (those
            # units complete mid-stream), folded t3 for rows 39..55 in
            # three shrinking chunks so only a 4-row copy+mul+DMA chain
            # trails the final matmul
            n = NPC - 1
            xc = xcs[n]
            t1s = t1pool.tile([C, FOLD0 + 2, W], BF16, name="t1s")
            nc.gpsimd.memset(t1s[:, 0:1, :], 0.0)
            t4s = t4pool.tile([C, H, W], BF16, name="t4s")
            ot = opool.tile([C, H, W], BF16, name="ot")

            def macmul_last(r0, r1):
                rr = r1 - r0
                sa = tmpool.tile([C, 24, W], BF16, name="sa")
                sb = tmpool.tile([C, 24, W], BF16, name="sb")
                sc = tmpool.tile([C, 24, W], BF16, name="sc")
                uu = tmpool.tile([C, 24, W], BF16, name="uu")
                t3 = tmpool.tile([C, 24, W], BF16, name="t3")
                nc.vector.tensor_scalar_mul(
                    sa[:, 0:rr, :], t1s[:, r0 : r0 + rr, :], w3_t[:, 0:1]
                )
                nc.vector.tensor_scalar_mul(
                    sb[:, 0:rr, :], t1s[:, r0 + 1 : r0 + 1 + rr, :], w3_t[:, 1:2]
                )
                nc.vector.tensor_scalar_mul(
                    sc[:, 0:rr, :], t1s[:, r0 + 2 : r0 + 2 + rr, :], w3_t[:, 2:3]
                )
                nc.vector.tensor_add(uu[:, 0:rr, :], sa[:, 0:rr, :], sb[:, 0:rr, :])
                nc.vector.tensor_add(t3[:, 0:rr, :], uu[:, 0:rr, :], sc[:, 0:rr, :])
                nc.gpsimd.tensor_mul(
                    ot[:, r0:r1, :], t3[:, 0:rr, :], t4s[:, r0:r1, :]
                )
                nc.sync.dma_start(out_d[n, :, r0:r1, :], ot[:, r0:r1, :])

            for c in range(5):  # t1 rows 0..39
                pa = papool.tile([C, FROWS, W], F32, name="pa")
                emit_mms(_t1_matmuls(c, pa, xc, w5_t))
                pb = pbpool.tile([C, CH, W], F32)
                emit_mms(_t4_matmuls(c, pb, xc, w4_t))
                nc.scalar.copy(
                    t1s[:, c * CH + 1 : c * CH + 1 + CH, :], pa[:, 0:CH, :]
                )
                nc.scalar.copy(t4s[:, c * CH : (c + 1) * CH, :], pb[:])
                if c == 3:
                    macmul_last(0, 24)
                if c == 4:
                    macmul_last(24, 39)
            for h0, h1 in ((39, 47), (47, 52), (52, 56)):
                pa = papool.tile([C, FROWS, W], F32, name="pa")
                emit_mms(_t3_fold_matmuls(pa, xc, wc_t, h0, h1 - h0))
                if h0 == 39:
                    pb = pbpool.tile([C, CH, W], F32)
                    emit_mms(_t4_matmuls(5, pb, xc, w4_t))
                    nc.scalar.copy(t4s[:, 40:48, :], pb[:])
                if h0 == 47:
                    pb = pbpool.tile([C, CH, W], F32)
                    emit_mms(_t4_matmuls(6, pb, xc, w4_t))
                    nc.scalar.copy(t4s[:, 48:H, :], pb[:])
                nc.vector.tensor_mul(
                    ot[:, h0:h1, :], pa[:, 0 : h1 - h0, :], t4s[:, h0:h1, :]
                )
                nc.sync.dma_start(out_d[n, :, h0:h1, :], ot[:, h0:h1, :])

    nc.compile()
    return nc


def _get_compiled():
    global _COMPILED
    if _COMPILED is None:
        _COMPILED = _build()
    return _COMPILED


def _prep_weights(w1, w3, w4):
    w1c = np.asarray(w1, dtype=np.float32)[:, :, :, 0]  # (co, ci, 5)
    w3c = np.asarray(w3, dtype=np.float32)[:, 0, :, 0]  # (co, 3)
    wc = np.zeros((C, 11, C), dtype=np.float32)         # (ci, tap, co)
    for d in range(3):
        for e in range(5):
            # wc[ci, d+e, co] += w1[co, ci, e] * w3[co, d]
            wc[:, d + e, :] += (w1c[:, :, e] * w3c[:, d][:, None]).T
    # border clip corrections (see _t3_fold_matmuls): taps 7,8 fix h=0;
    # taps 9,10 fix h=55
    for j, e in enumerate((3, 4)):
        wc[:, 7 + j, :] = -(w1c[:, :, e] * w3c[:, 0][:, None]).T
    for j, e in enumerate((0, 1)):
        wc[:, 9 + j, :] = -(w1c[:, :, e] * w3c[:, 2][:, None]).T
    wc5 = np.ascontiguousarray(w1c.transpose(1, 2, 0))  # (ci, tap, co)
    w4c = np.asarray(w4, dtype=np.float32)[:, :, 0, :]  # (ci, k, g)
    w4b = np.ascontiguousarray(np.tile(w4c, (1, 1, C // G)))  # (ci, k, 128)
    bf = ml_dtypes.bfloat16
    return (
        np.ascontiguousarray(wc).astype(bf),
        wc5.astype(bf),
        np.ascontiguousarray(w3c),
        w4b.astype(bf),
    )


def kernel(x, w1, w3, w4):
    global LAST_EXEC_NS, LAST_RESULTS
    nc = _get_compiled()
    xb = np.ascontiguousarray(np.asarray(x, dtype=np.float32)).astype(ml_dtypes.bfloat16)
    wc, wc5, w3c, w4b = _prep_weights(w1, w3, w4)

    in_maps = [
        {
            "x_s": np.ascontiguousarray(xb[i * NPC : (i + 1) * NPC]),
            "wc": wc,
            "wc5": wc5,
            "w3c": w3c,
            "w4b": w4b,
        }
        for i in range(NCORES)
    ]
    if TRACE:
        _enable_trace_hook()
    res = bass_utils.run_bass_kernel_spmd(
        nc,
        in_maps,
        core_ids=list(range(NCORES)),
        trace=TRACE,
        tmpdir=TRACE_DIR,
    )
    LAST_EXEC_NS = res.exec_time_ns
    LAST_RESULTS = res
    out = np.concatenate(
        [np.asarray(res.results[i]["out"]) for i in range(NCORES)], axis=0
    ).astype(np.float32)
    return out


# revision 29
# speedup vs baseline: 1.0833x; 1.0833x over previous
"""Trainium2 Bass kernel for dense_cnn problem.

Math (per batch element n, C=128 channels, H=W=56, G=8):
  t1 = conv_h(x, w1)          5-tap conv over H with full channel mixing
  t3 = dwconv_h(t1, w3)       3-tap depthwise conv over H
  t4[g] = sum_{c,k} x[c, h, w+2k-2] * w4[c,k,g]   (3 width taps, dil 2)
  out[c] = t3[c] * t4[c % 8]

Device strategy (data-parallel, 4 batch elems per core across 8 cores):
  - Fold t3 = w3 (*) w1 (*) x into ONE 7-tap H-conv with combined
    weights -> PE matmuls (plus 4 tiny correction matmuls for the rows
    where the fold wrongly includes clipped t1 values).  Measured DVE
    elementwise costs make this strictly better than computing the
    depthwise conv on the vector engines: the fold costs 2 extra PE
    taps (~373ns/chunk) vs ~2500ns/chunk of elementwise work.
  - w4 broadcast to 128 output channels on the host -> t4 computed at
    128 channels (3 PE matmuls), final combine elementwise.
  - Matmuls in bf16; accumulation fp32 in PSUM.  x unpadded in SBUF;
    boundary taps are clipped matmuls.
  - Per 8-row chunk: psA (t3) and psB (t4); ScalarE copies psB->SBUF
    (bf16), DVE multiplies psA (fp32 PSUM) against it writing the bf16
    output strip.  Output DMA'd bf16 (half the bytes of fp32) and
    upconverted on the host; one DMA per element, the last element
    split so only a single 8-row transfer trails the last multiply.
  - x loads for elements 1-3 are issued mid-stream: issuing them up
    front steals HBM bandwidth from the pieces the PE needs first, and
    the resulting matmul stalls also reset the PE_HAM activity window,
    delaying the 1.2 -> 2.4 GHz clock boost.
  - Small dummy matmuls (sized to end right as the first x piece and
    weights land) trip the PE_HAM clock gate while the first DMAs are
    still streaming in.
"""

import sys

sys.path.insert(0, "/opt/trn_rl_repo")

import ml_dtypes
import numpy as np

import concourse.bacc as bacc
import concourse.bass as bass
import concourse.mybir as mybir
import concourse.tile as tile
from concourse import bass_utils

N, C, H, W, G = 32, 128, 56, 56, 8
NCORES = 8
NPC = N // NCORES  # batch elems per core
CH = 8             # H rows per chunk (PSUM bank = 448 fp32 cols)
NCHUNK = H // CH

F32 = mybir.dt.float32
BF16 = mybir.dt.bfloat16

TRACE = False
TRACE_DIR = None
LAST_EXEC_NS = None
LAST_RESULTS = None

_COMPILED = None


def _enable_trace_hook():
    """The agent image's ``antenv`` lacks ``axon_hooks``, so the boot-time
    NTFF hook registration silently degraded. Recreate the module and
    register the same ctypes-based hook; also skip the bucket upload."""
    import sys as _sys
    import types

    if "antenv.axon_hooks" not in _sys.modules:
        mod = types.ModuleType("antenv.axon_hooks")
        mod._hook = None

        def set_axon_ntff_profile_hook(h):
            mod._hook = h

        def get_axon_ntff_profile_hook():
            return mod._hook

        mod.set_axon_ntff_profile_hook = set_axon_ntff_profile_hook
        mod.get_axon_ntff_profile_hook = get_axon_ntff_profile_hook
        _sys.modules["antenv.axon_hooks"] = mod
        import antenv

        antenv.axon_hooks = mod

    from antenv.axon_hooks import get_axon_ntff_profile_hook as _get

    if _get() is None:
        from trn_agent_boot.trn_boot import _ntff_profile_via_ctypes

        hook = _ntff_profile_via_ctypes("/opt/axon/libaxon_pjrt.so")
        if hook is not None:
            _sys.modules["antenv.axon_hooks"].set_axon_ntff_profile_hook(hook)

    bass_utils.upload_artifacts = lambda tmpdir: f"local:{tmpdir}"


FOLD0 = 47         # fold region rows FOLD0..H-1; t1/DVE covers 0..FOLD0-1
FROWS = H - FOLD0  # 9 rows -> 504 fp32 cols, fits one PSUM bank


def _t3_fold_matmuls(pa, xc, wc_t, h0, rows):
    """Folded 7-tap conv for t3 rows h0..h0+rows-1 (row-clipped at the H
    borders) plus the t1-clip correction taps where the region touches
    h=0 or h=H-1.  Output row o reads x row h0+o+f-3."""
    mms = []
    # f=3 covers the full chunk -> emitted first (start=True)
    for f in (3, 0, 1, 2, 4, 5, 6):
        o_lo = max(0, 3 - f - h0)
        o_hi = min(rows, H + 3 - f - h0)
        if o_lo >= o_hi:
            continue
        r0 = h0 + o_lo + f - 3
        r1 = h0 + o_hi + f - 3
        mms.append((wc_t[:, f, :], xc[:, r0:r1, :], pa[:, o_lo:o_hi, :]))
    if h0 == 0:
        # fold wrongly includes t1[-1] at h=0
        for j in range(2):
            mms.append((wc_t[:, 7 + j, :], xc[:, j : j + 1, :], pa[:, 0:1, :]))
    if h0 + rows == H:
        # fold wrongly includes t1[56] at h=55
        for j in range(2):
            mms.append(
                (wc_t[:, 9 + j, :], xc[:, 54 + j : 55 + j, :], pa[:, rows - 1 : rows, :])
            )
    return mms


def _t1_matmuls(c, pa, xc, w5_t):
    """5-tap t1 conv for the 8-row chunk c (t1 rows 8c..8c+7), clipped at
    the top border.  Tap e=2 covers the full chunk and is emitted first."""
    h0 = c * CH
    mms = []
    for e in (2, 0, 1, 3, 4):
        o_lo = max(0, 2 - e - h0)
        o_hi = min(CH, H + 2 - e - h0)
        if o_lo >= o_hi:
            continue
        r0 = h0 + o_lo + e - 2
        r1 = h0 + o_hi + e - 2
        mms.append((w5_t[:, e, :], xc[:, r0:r1, :], pa[:, o_lo:o_hi, :]))
    return mms


def _t4_matmuls(c, pb, xc, w4_t):
    """t4 chunk: 3 width taps at offsets -2/0/+2, col-clipped at borders."""
    h0 = c * CH
    rows = xc[:, h0 : h0 + CH, :]
    return [
        (w4_t[:, 1, :], rows, pb[:]),                               # delta = 0
        (w4_t[:, 0, :], xc[:, h0 : h0 + CH, 0 : W - 2], pb[:, :, 2:W]),   # -2
        (w4_t[:, 2, :], xc[:, h0 : h0 + CH, 2:W], pb[:, :, 0 : W - 2]),   # +2
    ]


def _build():
    nc = bacc.Bacc(
        "TRN2",
        target_bir_lowering=False,
        debug=False,
        enable_asserts=False,
        num_devices=NCORES,
    )

    x_d = nc.dram_tensor("x_s", (NPC, C, H, W), BF16, kind="ExternalInput").ap()
    wc_d = nc.dram_tensor("wc", (C, 11, C), BF16, kind="ExternalInput").ap()
    w5_d = nc.dram_tensor("wc5", (C, 5, C), BF16, kind="ExternalInput").ap()
    w4_d = nc.dram_tensor("w4b", (C, 3, C), BF16, kind="ExternalInput").ap()
    w3_d = nc.dram_tensor("w3c", (C, 3), F32, kind="ExternalInput").ap()
    out_d = nc.dram_tensor("out", (NPC, C, H, W), BF16, kind="ExternalOutput").ap()

    NT1 = FOLD0 // CH + 1  # 6 t1 chunks covering rows 0..47
    # depthwise units (out-row range, ready-after-t1-chunk)
    units_mid = ((0, 24, 3), (24, FOLD0, 5))

    with tile.TileContext(nc) as tc:
        with (
            tc.tile_pool(name="wpool", bufs=1) as wpool,
            tc.tile_pool(name="xpool", bufs=1) as xpool,
            tc.tile_pool(name="t1pool", bufs=2) as t1pool,
            tc.tile_pool(name="t4pool", bufs=2) as t4pool,
            tc.tile_pool(name="tmpool", bufs=2) as tmpool,
            tc.tile_pool(name="opool", bufs=2) as opool,
            tc.tile_pool(name="psA", bufs=4, space="PSUM") as papool,
            tc.tile_pool(name="psB", bufs=3, space="PSUM") as pbpool,
            tc.tile_pool(name="psD", bufs=1, space="PSUM") as pdpool,
        ):
            # Dummy matmuls on a zeroed SBUF strip while the first DMAs
            # stream in: PE_HAM ungates the 2.4 GHz clock only after
            # ~3.4us of sustained activity.  The garbage results go to a
            # PSUM bank that is never read.  memset on GpSimd (an
            # early-ready engine the PE never waits for).
            dmy = wpool.tile([C, 448], BF16)
            nc.gpsimd.memset(dmy[:], 0.0)
            dps = pdpool.tile([C, 448], F32)
            for _ in range(8):
                nc.tensor.matmul(
                    dps[:], lhsT=dmy[:, 0:C], rhs=dmy[:], start=True, stop=True
                )

            wc_t = wpool.tile([C, 11, C], BF16)
            w5_t = wpool.tile([C, 5, C], BF16)
            w4_t = wpool.tile([C, 3, C], BF16)
            w3_t = wpool.tile([C, 3], F32)

            xcs = []
            for n in range(NPC):
                xc = xpool.tile([C, H, W], BF16, name=f"xc{n}")
                xcs.append(xc)
            # weights + first batch elem first (in pieces, so chunk-0
            # matmuls start early); later elems' loads are issued from
            # inside the chunk loop
            nc.sync.dma_start(w5_t[:], w5_d[:])
            nc.sync.dma_start(xcs[0][:, 0:18, :], x_d[0, :, 0:18, :])
            nc.sync.dma_start(w4_t[:], w4_d[:])
            nc.sync.dma_start(w3_t[:], w3_d[:])
            nc.sync.dma_start(xcs[0][:, 18:34, :], x_d[0, :, 18:34, :])
            nc.sync.dma_start(xcs[0][:, 34:H, :], x_d[0, :, 34:H, :])
            nc.sync.dma_start(wc_t[:], wc_d[:])

            def emit_mms(mms):
                for i, (lhsT, rhs, outap) in enumerate(mms):
                    nc.tensor.matmul(
                        outap, lhsT=lhsT, rhs=rhs,
                        start=(i == 0), stop=(i == len(mms) - 1),
                    )

            # elements 0..NPC-2: hybrid -- plain t1 chunks for rows
            # 0..FOLD0-1 with the depthwise conv on DVE (mid-stream,
            # plenty of pipeline room), folded t3 for the last rows
            for n in range(NPC - 1):
                xc = xcs[n]

                # t1 strip rows 0..48: strip row r holds t1 row r-1; row 0
                # is the depthwise conv's zero padding
                t1s = t1pool.tile([C, FOLD0 + 2, W], BF16, name="t1s")
                nc.gpsimd.memset(t1s[:, 0:1, :], 0.0)
                t4s = t4pool.tile([C, H, W], BF16, name="t4s")
                ot = opool.tile([C, H, W], BF16, name="ot")

                def macmul(unit):
                    # t3 rows r0..r1-1 = sum of three per-partition-scaled
                    # shifted t1 reads (tensor_scalar hits the DVE fast
                    # bf16 mode; scalar_tensor_tensor would not), then the
                    # combine against the t4 strip on the Pool engine
                    r0, r1, _ = unit
                    rr = r1 - r0
                    sa = tmpool.tile([C, 24, W], BF16, name="sa")
                    sb = tmpool.tile([C, 24, W], BF16, name="sb")
                    sc = tmpool.tile([C, 24, W], BF16, name="sc")
                    uu = tmpool.tile([C, 24, W], BF16, name="uu")
                    t3 = tmpool.tile([C, 24, W], BF16, name="t3")
                    nc.vector.tensor_scalar_mul(
                        sa[:, 0:rr, :], t1s[:, r0 : r0 + rr, :], w3_t[:, 0:1]
                    )
                    nc.vector.tensor_scalar_mul(
                        sb[:, 0:rr, :], t1s[:, r0 + 1 : r0 + 1 + rr, :], w3_t[:, 1:2]
                    )
                    nc.vector.tensor_scalar_mul(
                        sc[:, 0:rr, :], t1s[:, r0 + 2 : r0 + 2 + rr, :], w3_t[:, 2:3]
                    )
                    nc.vector.tensor_add(uu[:, 0:rr, :], sa[:, 0:rr, :], sb[:, 0:rr, :])
                    nc.vector.tensor_add(t3[:, 0:rr, :], uu[:, 0:rr, :], sc[:, 0:rr, :])
                    nc.gpsimd.tensor_mul(
                        ot[:, r0:r1, :], t3[:, 0:rr, :], t4s[:, r0:r1, :]
                    )

                for c in range(NT1):
                    # single shape/name so U-chunks and the fold chunk
                    # share one 3-buffer PSUM rotation (9 rows <= 1 bank)
                    pa = papool.tile([C, FROWS, W], F32, name="pa")
                    emit_mms(_t1_matmuls(c, pa, xc, w5_t))
                    pb = pbpool.tile([C, CH, W], F32)
                    emit_mms(_t4_matmuls(c, pb, xc, w4_t))
                    # later elements' x loads, issued mid-stream (before
                    # the out-DMA issues: SyncE executes in order and
                    # the out issues block on compute semaphores)
                    if (n, c) in ((0, 2), (0, 5), (1, 5)):
                        k = n + 1 if c == 2 else n + 2
                        nc.sync.dma_start(xcs[k][:], x_d[k])
                    # PSUM chunks -> bf16 SBUF strips on ScalarE
                    nc.scalar.copy(
                        t1s[:, c * CH + 1 : c * CH + 1 + CH, :], pa[:, 0:CH, :]
                    )
                    nc.scalar.copy(t4s[:, c * CH : (c + 1) * CH, :], pb[:])
                    for unit in units_mid:
                        if unit[2] == c:
                            macmul(unit)

                # fold region rows FOLD0..H-1: 7-tap folded t3 in one
                # PSUM chunk, multiplied directly against the t4 strip
                paf = papool.tile([C, FROWS, W], F32, name="pa")
                emit_mms(_t3_fold_matmuls(paf, xc, wc_t, FOLD0, FROWS))
                pb = pbpool.tile([C, CH, W], F32)
                emit_mms(_t4_matmuls(NCHUNK - 1, pb, xc, w4_t))
                nc.scalar.copy(t4s[:, (NCHUNK - 1) * CH : H, :], pb[:])
                nc.vector.tensor_mul(
                    ot[:, FOLD0:H, :], paf[:], t4s[:, FOLD0:H, :]
                )
                nc.sync.dma_start(out_d[n], ot[:])

            # last element: fully folded -- no depthwise work can trail
            # the matmul stream (the DVE/Pool queues run several us
            # behind their emission points by stream end, so any
            # late-emitted unit work lands squarely in the tail).  The
            # final fold chunk's multiply is split 4+4 rows so only a
            # short copy+mul+DMA chain trails the last matmul.
            n = NPC - 1
            xc = xcs[n]
            ot = opool.tile([C, H, W], BF16, name="ot")
            for c in range(NCHUNK):
                pa = papool.tile([C, FROWS, W], F32, name="pa")
                emit_mms(_t3_fold_matmuls(pa, xc, wc_t, c * CH, CH))
                pb = pbpool.tile([C, CH, W], F32)
                emit_mms(_t4_matmuls(c, pb, xc, w4_t))
                t4c = t4pool.tile([C, CH, W], BF16, name="t4c")
                nc.scalar.copy(t4c[:], pb[:])
                if c < NCHUNK - 1:
                    nc.vector.tensor_mul(
                        ot[:, c * CH : (c + 1) * CH, :], pa[:, 0:CH, :], t4c[:]
                    )
                    if c == NCHUNK - 2:
                        nc.sync.dma_start(
                            out_d[n, :, 0 : (c + 1) * CH, :],
                            ot[:, 0 : (c + 1) * CH, :],
                        )
                else:
                    h0 = c * CH
                    for r0, r1 in ((0, 4), (4, CH)):
                        nc.vector.tensor_mul(
                            ot[:, h0 + r0 : h0 + r1, :],
                            pa[:, r0:r1, :],
                            t4c[:, r0:r1, :],
                        )
                        nc.sync.dma_start(
                            out_d[n, :, h0 + r0 : h0 + r1, :],
                            ot[:, h0 + r0 : h0 + r1, :],
                        )

    nc.compile()
    return nc


def _get_compiled():
    global _COMPILED
    if _COMPILED is None:
        _COMPILED = _build()
    return _COMPILED


def _prep_weights(w1, w3, w4):
    w1c = np.asarray(w1, dtype=np.float32)[:, :, :, 0]  # (co, ci, 5)
    w3c = np.asarray(w3, dtype=np.float32)[:, 0, :, 0]  # (co, 3)
    wc = np.zeros((C, 11, C), dtype=np.float32)         # (ci, tap, co)
    for d in range(3):
        for e in range(5):
            # wc[ci, d+e, co] += w1[co, ci, e] * w3[co, d]
            wc[:, d + e, :] += (w1c[:, :, e] * w3c[:, d][:, None]).T
    # border clip corrections (see _t3_fold_matmuls): taps 7,8 fix h=0;
    # taps 9,10 fix h=55
    for j, e in enumerate((3, 4)):
        wc[:, 7 + j, :] = -(w1c[:, :, e] * w3c[:, 0][:, None]).T
    for j, e in enumerate((0, 1)):
        wc[:, 9 + j, :] = -(w1c[:, :, e] * w3c[:, 2][:, None]).T
    wc5 = np.ascontiguousarray(w1c.transpose(1, 2, 0))  # (ci, tap, co)
    w4c = np.asarray(w4, dtype=np.float32)[:, :, 0, :]  # (ci, k, g)
    w4b = np.ascontiguousarray(np.tile(w4c, (1, 1, C // G)))  # (ci, k, 128)
    bf = ml_dtypes.bfloat16
    return (
        np.ascontiguousarray(wc).astype(bf),
        wc5.astype(bf),
        np.ascontiguousarray(w3c),
        w4b.astype(bf),
    )


def kernel(x, w1, w3, w4):
    global LAST_EXEC_NS, LAST_RESULTS
    nc = _get_compiled()
    xb = np.ascontiguousarray(np.asarray(x, dtype=np.float32)).astype(ml_dtypes.bfloat16)
    wc, wc5, w3c, w4b = _prep_weights(w1, w3, w4)

    in_maps = [
        {
            "x_s": np.ascontiguousarray(xb[i * NPC : (i + 1) * NPC]),
            "wc": wc,
            "wc5": wc5,
            "w3c": w3c,
            "w4b": w4b,
        }
        for i in range(NCORES)
    ]
    if TRACE:
        _enable_trace_hook()
    res = bass_utils.run_bass_kernel_spmd(
        nc,
        in_maps,
        core_ids=list(range(NCORES)),
        trace=TRACE,
        tmpdir=TRACE_DIR,
    )
    LAST_EXEC_NS = res.exec_time_ns
    LAST_RESULTS = res
    out = np.concatenate(
        [np.asarray(res.results[i]["out"]) for i in range(NCORES)], axis=0
    ).astype(np.float32)
    return out
